# revision 41
# baseline (speedup 1.0000x reference)
"""Trainium2 Bass kernel for nn_ExpressionModel (dense DiT-style transformer block).

Sharding: 8 cores = 2 (batch) x 4 (sequence chunks of 512 tokens).
Each core computes the full block for its 512 query tokens; K/V projections
for the full 2048-token batch are duplicated across the 4 cores of a batch
(no collectives needed).

Residual stream is transposed (channels on partitions). All dense
projections run in fp8e4 with DoubleRow perf mode (two contraction rows per
PE pass); the MLP uses hi+lo fp8 splitting (T ~ T_hi + T_lo/64) for both
weights and activations on gate/up, and for weights on down, to stay inside
the error budget. Attention scores / probabilities / p@V stay bf16.
RoPE is computed from two projections (natural + host-swapped weights) so
no engine shuffles partitions: k_rope = pk*cos + pks*sin_signed — two DVE
muls (PSUM direct) + one Pool add. adaLN runs weight-stationary (1-column
matmuls, ~free on PE); only shift/scale_sa loads up front, the other 32
columns stream in during self-attention.
"""

import numpy as np
import ml_dtypes

import concourse.bass as bass
import concourse.tile as tile
from concourse import bacc, mybir
from concourse.bass_utils import run_bass_kernel_spmd

FP32 = mybir.dt.float32
BF16 = mybir.dt.bfloat16
F8 = mybir.dt.float8e4
DR = mybir.MatmulPerfMode.DoubleRow
F8NP = ml_dtypes.float8_e4m3

STAGE_MARKS = []  # (instruction-id watermark, stage name) — profiling aid

B, L, C = 2, 2048, 1024
H, D = 16, 64
L2, TD = 512, 768
FF = 4096
EPS = 1e-6
NCORE = 8
LQ = 512            # query tokens per core
CT = C // 128       # 8 C partition-tiles
KP = C // 256       # 4 DoubleRow contraction pairs over C
LKT = L // 128      # 16 key chunks (self)
LCH = L // 512      # 4 512-token chunks
KSC = 1.0 / 8.0     # 1/sqrt(D)
LOSC = 64.0         # hi/lo split scale


def build_bass():
    nc = bacc.Bacc("TRN2", target_bir_lowering=False, debug=False)
    STAGE_MARKS.clear()

    def mark(stage):
        STAGE_MARKS.append((nc.next_id(), stage))

    def dma(out, in_):
        return nc.sync.dma_start(out=out, in_=in_)

    def din(name, shape, dt):
        return nc.dram_tensor(name, list(shape), dt, kind="ExternalInput")

    # --- inputs ---
    x_bf = din("x_bf", (C, L), BF16)            # x[b].T, bf16
    xq_f = din("xq_f", (C, LQ), FP32)           # own-chunk x[b].T, fp32 residual
    aud2 = din("aud2", (128, 3, 2, L2), F8)     # audio.T fp8 DR-paired
    cst = din("cst", (128, 80), FP32)    # tmod|adab|n1|n2|n3
    cs4 = din("cs4", (128, L), BF16)            # [c;c;c;c] rows
    sc4 = din("sc4", (128, L), BF16)            # [-s;+s;-s;+s] rows
    wadaA = din("wadaA", (128, CT, 2048), BF16)   # adaLN W cols j0..15
    wadaB = din("wadaB", (8, 128, CT, 512), BF16)  # adaLN W cols j16..47, 8 pieces
    wq2 = din("wq2", (128, KP, 2, C), F8)       # W_qkv q block, rope-permuted, DR-paired
    wqs2 = din("wqs2", (128, KP, 2, C), F8)     # q block, swap-permuted
    wk2 = din("wk2", (128, KP, 2, C), F8)
    wks2 = din("wks2", (128, KP, 2, C), F8)
    wv2 = din("wv2", (128, KP, 2, C), F8)
    wsa2 = din("wsa2", (128, KP, 2, C), F8)
    wqc2 = din("wqc2", (128, KP, 2, C), F8)
    wkv2 = din("wkv2", (128, 3, 2, 2 * C), F8)
    wca2 = din("wca2", (128, KP, 2, C), F8)
    wgh = din("wgh", (8, 128, KP, 2, 512), F8)  # MLP weights hi/lo fp8
    wgl = din("wgl", (8, 128, KP, 2, 512), F8)
    wuh = din("wuh", (8, 128, KP, 2, 512), F8)
    wul = din("wul", (8, 128, KP, 2, 512), F8)
    wdh = din("wdh", (CT, 128, 16, 2, 128), F8)   # W_down hi, per out C-tile
    wdl = din("wdl", (CT, 128, 16, 2, 128), F8)

    outT = nc.dram_tensor("outT", [C, LQ], FP32, kind="ExternalOutput")

    with tile.TileContext(nc) as tc:
        with (
            tc.tile_pool(name="pp", bufs=1) as pp,              # persistent
            tc.tile_pool(name="ps", bufs=1, space="PSUM") as ps,
        ):
            # ---- persistent constants (one packed tile) ----
            c_all = pp.tile([128, 80], FP32, tag="c_all")
            c_tmod = c_all[:, 0:CT]
            c_adab = c_all[:, 8:56]
            c_n1 = c_all[:, 56:64]
            c_n2 = c_all[:, 64:72]
            c_n3 = c_all[:, 72:80]
            c_cs4 = pp.tile([128, L], BF16, tag="c_cs4")
            c_sc4 = pp.tile([128, L], BF16, tag="c_sc4")
            xres = pp.tile([128, CT, LQ], FP32, tag="xres")
            ones_col = pp.tile([128, 1], BF16, tag="ones_col")
            ones_row = pp.tile([1, 128], BF16, tag="ones_row")
            eps_c = pp.tile([1, 1], FP32, tag="eps_c")
            nc.gpsimd.memset(ones_col, 1.0)
            nc.gpsimd.memset(ones_row, 1.0)
            nc.gpsimd.memset(eps_c, EPS)
            modsT = pp.tile([128, 48], FP32, tag="modsT")
            silu_bf = pp.tile([128, CT], BF16, tag="silu_bf")
            w1eff = pp.tile([128, CT], FP32, tag="w1eff")
            w3eff = pp.tile([128, CT], FP32, tag="w3eff")
            # attn output accumulators (fp8, DR-paired; reused by cross attn)
            att2 = [pp.tile([128, 2, LQ], F8, tag=f"att{j}", name=f"att{j}")
                    for j in range(KP)]
            # cross K (transposed) / V (natural), filled during self-attn
            kcT = [pp.tile([128, L2], BF16, tag=f"kc{m}", name=f"kcT{m}")
                   for m in range(CT)]
            vcb = [pp.tile([128, H, D + 1], BF16, tag=f"vc{t}", name=f"vcb{t}")
                   for t in range(4)]

            def sh_sa(k):
                return modsT[:, 0 + k:1 + k]

            def g_sa(k):
                return modsT[:, 16 + k:17 + k]

            def sh_ml(k):
                return modsT[:, 24 + k:25 + k]

            def g_ml(k):
                return modsT[:, 40 + k:41 + k]

            with tc.tile_pool(name="pkv", bufs=1) as pkv:
                vsb = [pkv.tile([128, H, D + 1], BF16, tag=f"v{t}", name=f"v{t}")
                       for t in range(LKT)]
                qT = [pkv.tile([128, LQ], BF16, tag=f"qT{m}", name=f"qT{m}")
                      for m in range(CT)]
                # adaLN-A weights borrow the kT buffers (kT unused until k_proj)
                wadaA_t = [pkv.tile([128, L], BF16, tag="kTx", bufs=8,
                                    name=f"wadaA{i}") for i in range(8)]

                with tc.tile_pool(name="pqw", bufs=1) as pqw:
                    # qkv weights: q/k rotate one buffer, swaps likewise
                    w_q = pqw.tile([128, KP, 2, C], F8, tag="wmain", bufs=2, name="w_q")
                    w_qs = pqw.tile([128, KP, 2, C], F8, tag="wswap", bufs=1, name="w_qs")
                    xsa2 = [pqw.tile([128, 2, L], F8, tag=f"xsa{j}", name=f"xsa{j}")
                            for j in range(KP)]
                    # streamed x (4 chunks, 2 resident) and adaLN-A (2 pieces)
                    xc = {}

                    def x_fetch(lc, q=None):
                        xc[lc] = pqw.tile([128, CT, 512], BF16, tag="xinc",
                                          bufs=3, name=f"xin{lc}")
                        (q or nc.sync).dma_start(out=xc[lc], in_=x_bf[:, :].rearrange(
                            "(k p) l -> p k l", p=128)[:, :, lc * 512:(lc + 1) * 512])

                    # ---- DMA issue order (SP FIFO) ----
                    x_fetch(0)
                    dma(out=c_all, in_=cst[:, :])
                    x_fetch(1)
                    x_fetch(2)
                    for i in range(8):
                        dma(out=wadaA_t[i], in_=wadaA[:, :, i * 256:(i + 1) * 256])
                    dma(out=c_cs4, in_=cs4[:, :])
                    dma(out=c_sc4, in_=sc4[:, :])
                    dma(out=w_q, in_=wq2[:, :, :, :])
                    dma(out=w_qs, in_=wqs2[:, :, :, :])

                    mark("norm1")
                    # ---- silu(t_mod) ----
                    sg_t = pqw.tile([128, CT], FP32, tag="sg_t")
                    nc.scalar.activation(out=sg_t, in_=c_tmod,
                                         func=mybir.ActivationFunctionType.Sigmoid)
                    nc.vector.tensor_mul(silu_bf, sg_t, c_tmod)

                    pbs = {}

                    def norm1_ssq(lc):
                        pssq = ps.tile([1, 512], FP32, tag="pC", bufs=2,
                                       name=f"pssq{lc}")
                        for k in range(CT):
                            xsq = pqw.tile([128, 512], BF16, tag="xsq", bufs=1,
                                           name=f"xsq{lc}_{k}")
                            nc.vector.tensor_mul(xsq, xc[lc][:, k, :], xc[lc][:, k, :])
                            nc.tensor.matmul(pssq, ones_col, xsq,
                                             start=(k == 0), stop=(k == CT - 1))
                        rstd = pqw.tile([1, 512], FP32, tag="rstd", bufs=1,
                                        name=f"rstd{lc}")
                        nc.scalar.activation(out=rstd, in_=pssq,
                                             func=mybir.ActivationFunctionType.Sqrt,
                                             bias=eps_c, scale=1.0 / C)
                        nc.vector.reciprocal(rstd, rstd)
                        rstd_bf = pqw.tile([1, 512], BF16, tag="rstd_bf", bufs=1,
                                           name=f"rstdb{lc}")
                        nc.vector.tensor_copy(rstd_bf, rstd)
                        pb = ps.tile([128, 512], FP32, tag="pA", bufs=2,
                                     name=f"pbn1{lc}")
                        nc.tensor.matmul(pb, ones_row, rstd_bf, start=True, stop=True)
                        pbsb = pqw.tile([128, 512], BF16, tag="pbsb", bufs=2,
                                        name=f"pbsb{lc}")
                        nc.scalar.copy(out=pbsb, in_=pb)
                        pbs[lc] = pbsb

                    def mod1(lc):
                        sl = slice(lc * 512, (lc + 1) * 512)
                        for k in range(CT):
                            dst = xsa2[k // 2][:, k % 2, sl]
                            nc.vector.scalar_tensor_tensor(
                                out=dst, in0=xc[lc][:, k, :],
                                scalar=w1eff[:, k:k + 1], in1=pbs[lc],
                                op0=mybir.AluOpType.mult,
                                op1=mybir.AluOpType.mult)
                            if k % 2 == 0:
                                nc.scalar.activation(
                                    out=dst, in_=dst,
                                    func=mybir.ActivationFunctionType.Identity,
                                    bias=sh_sa(k))
                            else:
                                nc.gpsimd.tensor_scalar(
                                    out=dst, in0=dst, scalar1=sh_sa(k),
                                    scalar2=None, op0=mybir.AluOpType.add)

                    norm1_ssq(0)
                    norm1_ssq(1)
                    norm1_ssq(2)

                    mark("adaLN")
                    # ---- adaLN part A: shift_sa + scale_sa (weight-stationary) ----
                    pmA = ps.tile([128, 16], FP32, tag="pC", bufs=2, name="pmA")
                    for j in range(16):
                        for k in range(CT):
                            nc.tensor.matmul(pmA[:, j:j + 1],
                                             wadaA_t[j // 2][:, k * 256 + (j % 2) * 128:
                                                             k * 256 + (j % 2) * 128 + 128],
                                             silu_bf[:, k:k + 1],
                                             start=(k == 0), stop=(k == CT - 1))
                    nc.vector.tensor_add(modsT[:, 0:16], pmA, c_adab[:, 0:16])
                    nc.vector.tensor_scalar(out=w1eff, in0=modsT[:, 8:16],
                                            scalar1=1.0, scalar2=None,
                                            op0=mybir.AluOpType.add)
                    nc.vector.tensor_mul(w1eff, w1eff, c_n1)

                    mark("mod1")
                    # ---- modulate -> xsa2 fp8 DR-paired ----
                    mod1(0)
                    x_fetch(3, nc.scalar)

                    def proj_dr(out_psum, w, m, xcols, nkp=KP):
                        for kp in range(nkp):
                            nc.tensor.matmul(out_psum,
                                             w[:, kp, :, m * 128:(m + 1) * 128],
                                             xcols(kp),
                                             start=(kp == 0), stop=(kp == nkp - 1),
                                             perf_mode=DR)

                    kT = [pkv.tile([128, L], BF16, tag="kTx", bufs=8,
                                   name=f"kT{m}") for m in range(CT)]
                    rope_rr = [0]

                    def rope_apply(dst, pk_, pks_, cols):
                        kb = pp.tile([128, 512], BF16, tag="ropet", bufs=6, name="kb")
                        nc.scalar.copy(out=kb, in_=pk_)
                        m1 = pp.tile([128, 512], BF16, tag="ropet", bufs=6, name="m1")
                        nc.vector.tensor_mul(m1, kb, c_cs4[:, cols])
                        m2 = pp.tile([128, 512], BF16, tag="ropet", bufs=6, name="m2")
                        nc.vector.tensor_mul(m2, pks_, c_sc4[:, cols])
                        rope_rr[0] ^= 1
                        if rope_rr[0]:
                            nc.gpsimd.tensor_add(dst, m1, m2)
                        else:
                            nc.vector.tensor_add(dst, m1, m2)

                    mark("q_proj")
                    # =========== q projection (own chunk = mod chunk 0) + rope ===========
                    OWN = slice(0, LQ)
                    for m in range(CT):
                        pq2 = ps.tile([128, 2 * LQ], FP32, tag="pQ", bufs=2,
                                      name=f"pq{m}")
                        proj_dr(pq2[:, 0:LQ], w_q, m, lambda kp: xsa2[kp][:, :, OWN])
                        proj_dr(pq2[:, LQ:2 * LQ], w_qs, m,
                                lambda kp: xsa2[kp][:, :, OWN])
                        rope_apply(qT[m], pq2[:, 0:LQ], pq2[:, LQ:2 * LQ], OWN)
                        if m == 0:
                            mod1(1)
                        if m == 2:
                            mod1(2)
                        if m == 4:
                            norm1_ssq(3)
                        if m == 6:
                            mod1(3)

                    mark("k_proj")
                    # ===== k projection (full L) + rope, v units interleaved =====
                    w_k = pqw.tile([128, KP, 2, C], F8, tag="wmain", bufs=2, name="w_k")
                    w_ks = pqw.tile([128, KP, 2, C], F8, tag="wswap", bufs=1, name="w_ks")
                    w_v = pqw.tile([128, KP, 2, C], F8, tag="wmain", bufs=2, name="w_v")
                    dma(out=w_k, in_=wk2[:, :, :, :])
                    dma(out=w_ks, in_=wks2[:, :, :, :])
                    dma(out=w_v, in_=wv2[:, :, :, :])
                    dma(out=xres, in_=xq_f[:, :].rearrange(
                        "(k p) l -> p k l", p=128))

                    def v_unit(t, g):
                        if g == 0:
                            nc.vector.memset(vsb[t][:, :, D:D + 1], 1.0)
                        pv = ps.tile([128, 512], FP32, tag="pA", bufs=2,
                                     name=f"pv{t}_{g}")
                        for kp in range(KP):
                            nc.tensor.matmul(
                                pv, xsa2[kp][:, :, t * 128:(t + 1) * 128],
                                w_v[:, kp, :, g * 512:(g + 1) * 512],
                                start=(kp == 0), stop=(kp == KP - 1),
                                perf_mode=DR)
                        nc.scalar.copy(
                            out=vsb[t][:, g * 8:(g + 1) * 8, 0:D],
                            in_=pv.rearrange("p (h d) -> p h d", h=8))

                    vq = [(t, g) for t in range(LKT) for g in range(2)]
                    vi = [0]
                    for m in range(CT):
                        for lc in range(LCH):
                            sl = slice(lc * 512, (lc + 1) * 512)
                            pk2 = ps.tile([128, 1024], FP32, tag="pQ", bufs=2,
                                          name=f"pk{m}_{lc}")
                            proj_dr(pk2[:, 0:512], w_k, m,
                                    lambda kp: xsa2[kp][:, :, sl])
                            proj_dr(pk2[:, 512:1024], w_ks, m,
                                    lambda kp: xsa2[kp][:, :, sl])
                            rope_apply(kT[m][:, sl], pk2[:, 0:512],
                                       pk2[:, 512:1024], sl)
                            if m >= 1:
                                t, g = vq[vi[0]]; vi[0] += 1
                                v_unit(t, g)

                    mark("v_proj")
                    while vi[0] < len(vq):
                        t, g = vq[vi[0]]; vi[0] += 1
                        v_unit(t, g)

                # pqw closed: qkv weights + xsa2 freed
                # weights/data needed during + after self-attn
                pat_cm = tc.tile_pool(name="pat", bufs=1)
                pat = pat_cm.__enter__()
                w_sa = pat.tile([128, KP, 2, C], F8, tag="w_sa")
                w_kv = pat.tile([128, 3, 2, 2 * C], F8, tag="w_kv")
                a_t = pat.tile([128, 3, 2, L2], F8, tag="a_t")
                dma(out=w_sa, in_=wsa2[:, :, :, :])
                dma(out=w_kv, in_=wkv2[:, :, :, :])
                dma(out=a_t, in_=aud2[:, :, :, :])
                wadaB_t = {}

                def adaB_fetch(i):
                    wadaB_t[i] = pat.tile([128, CT, 512], BF16, tag="wadaB",
                                          bufs=4, name=f"wadaB{i}")
                    dma(out=wadaB_t[i], in_=wadaB[i])

                def cross_kv_piece(i):
                    # i in 0..11: 8 kc tiles then 4 vc tiles
                    if i < 8:
                        m = i
                        pkc = ps.tile([128, L2], FP32, tag="pA", bufs=2, name=f"pkc{m}")
                        for kp in range(3):
                            nc.tensor.matmul(pkc,
                                             w_kv[:, kp, :, m * 128:(m + 1) * 128],
                                             a_t[:, kp, :, :],
                                             start=(kp == 0), stop=(kp == 2),
                                             perf_mode=DR)
                        nc.vector.tensor_copy(kcT[m], pkc)
                    else:
                        t = i - 8
                        nc.vector.memset(vcb[t][:, :, D:D + 1], 1.0)
                        for g in range(2):
                            pvc = ps.tile([128, 512], FP32, tag="pA", bufs=2,
                                          name=f"pvc{t}_{g}")
                            for kp in range(3):
                                nc.tensor.matmul(
                                    pvc, a_t[:, kp, :, t * 128:(t + 1) * 128],
                                    w_kv[:, kp, :, C + g * 512:C + (g + 1) * 512],
                                    start=(kp == 0), stop=(kp == 2),
                                    perf_mode=DR)
                            nc.vector.tensor_copy(
                                vcb[t][:, g * 8:(g + 1) * 8, 0:D],
                                pvc.rearrange("p (h d) -> p h d", h=8))

                def adaB_piece(i):
                    # modsT cols 16+4i .. 20+4i
                    j0 = 16 + 4 * i
                    pmB = ps.tile([128, 4], FP32, tag="pA", bufs=2, name=f"pmB{i}")
                    for jj in range(4):
                        for k in range(CT):
                            nc.tensor.matmul(pmB[:, jj:jj + 1],
                                             wadaB_t[i][:, k, jj * 128:(jj + 1) * 128],
                                             silu_bf[:, k:k + 1],
                                             start=(k == 0), stop=(k == CT - 1))
                    nc.vector.tensor_add(modsT[:, j0:j0 + 4], pmB,
                                         c_adab[:, j0:j0 + 4])
                    if i == 5:
                        nc.vector.tensor_scalar(out=w3eff, in0=modsT[:, 32:40],
                                                scalar1=1.0, scalar2=None,
                                                op0=mybir.AluOpType.add)
                        nc.vector.tensor_mul(w3eff, w3eff, c_n3)

                mark("self_attn")
                # =========== self-attention (software-pipelined stream) ===========
                # stream of (h, t) items; po lags LAG items behind its exp so
                # PE never blocks on Act, and head boundaries overlap.
                LAG = 5
                pos = {}
                pexps = {}
                pending = []  # (due_item, closure) in issue order

                def sa_epilogue(h):
                    def run():
                        m = h // 2
                        rs = slice((h % 2) * 64, (h % 2) * 64 + 64)
                        po = pos.pop(h)
                        rec = pp.tile([1, LQ], FP32, tag="rec", bufs=2,
                                      name=f"rec{h}")
                        nc.vector.reciprocal(rec, po[64:65, :])
                        rec_bf = pp.tile([1, LQ], BF16, tag="rec_bf", bufs=2,
                                         name=f"recb{h}")
                        nc.vector.tensor_copy(rec_bf, rec)
                        pbc = ps.tile([64, LQ], FP32, tag="pA", bufs=2,
                                      name=f"pbc{h}")
                        nc.tensor.matmul(pbc, ones_row[:, 0:64], rec_bf,
                                         start=True, stop=True)
                        rb_sb = pp.tile([64, LQ], BF16, tag="rb_sb", bufs=2,
                                        name=f"rb{h}")
                        nc.vector.tensor_copy(rb_sb, pbc)
                        nc.vector.tensor_mul(att2[m // 2][rs, m % 2, :],
                                             po[0:64, :], rb_sb)
                    return run

                def sa_po(h, tp):
                    def run():
                        px = pexps.pop((h, tp))
                        for t in (2 * tp, 2 * tp + 1):
                            nc.tensor.matmul(pos[h], vsb[t][:, h, :],
                                             px[:, (t % 2) * LQ:(t % 2 + 1) * LQ],
                                             start=(t == 0), stop=(t == LKT - 1))
                    return run

                NTP = LKT // 2
                NIT = H * NTP
                for g in range(NIT + NTP):
                    while pending and pending[0][0] <= g:
                        pending.pop(0)[1]()
                    if g >= NIT:
                        continue
                    h, tp = divmod(g, NTP)
                    m = h // 2
                    rs = slice((h % 2) * 64, (h % 2) * 64 + 64)
                    if tp == 0:
                        pos[h] = ps.tile([65, LQ], FP32, tag="pC", bufs=2,
                                         name=f"po{h}")
                        if h < 8:
                            adaB_fetch(h)
                    psc = ps.tile([128, 2 * LQ], FP32, tag="pQ", bufs=2,
                                  name=f"psc{h}_{tp}")
                    for t in (2 * tp, 2 * tp + 1):
                        nc.tensor.matmul(psc[:, (t % 2) * LQ:(t % 2 + 1) * LQ],
                                         kT[m][rs, t * 128:(t + 1) * 128],
                                         qT[m][rs, :], start=True, stop=True)
                    pexp = pat.tile([128, 2 * LQ], BF16, tag="pexpS", bufs=6,
                                    name=f"pexp{h}_{tp}")
                    nc.scalar.activation(out=pexp, in_=psc,
                                         func=mybir.ActivationFunctionType.Exp,
                                         scale=KSC)
                    pexps[(h, tp)] = pexp
                    pending.append((g + LAG, sa_po(h, tp)))
                    if tp == NTP - 1:
                        pending.append((g + LAG + 2, sa_epilogue(h)))
                        if 2 <= h < 14:
                            pending.append((g + LAG + 3, (lambda hh:
                                lambda: cross_kv_piece(hh - 2))(h)))
                        if h >= 8:
                            pending.append((g + LAG + 4, (lambda hh:
                                lambda: adaB_piece(hh - 8))(h)))
                while pending:
                    pending.pop(0)[1]()

                mark("sa_out")
                # =========== self-attn out proj + gated residual + norm2 ssq ===========
                pssq_n2 = ps.tile([1, LQ], FP32, tag="pC", bufs=2, name="pssq_n2")

                def n2_ssq(m):
                    xsq = pp.tile([128, LQ], BF16, tag="rb_sb", bufs=2,
                                  name=f"xsqn2_{m}")
                    nc.vector.tensor_mul(xsq, xres[:, m, :], xres[:, m, :])
                    nc.tensor.matmul(pssq_n2, ones_col, xsq,
                                     start=(m == 0), stop=(m == CT - 1))

                for m in range(CT):
                    pso = ps.tile([128, LQ], FP32, tag="pA", bufs=2, name=f"pso{m}")
                    proj_dr(pso, w_sa, m, lambda kp: att2[kp][:, :, :])
                    nc.vector.scalar_tensor_tensor(
                        out=xres[:, m, :], in0=pso, scalar=g_sa(m), in1=xres[:, m, :],
                        op0=mybir.AluOpType.mult, op1=mybir.AluOpType.add)
                    if m >= 2:
                        n2_ssq(m - 2)
                n2_ssq(CT - 2)
                n2_ssq(CT - 1)
                pat_cm.__exit__(None, None, None)

            mark("cross")
            # =========== cross attention + MLP ===========
            with tc.tile_pool(name="pca", bufs=1) as pca:
                w_qc = pca.tile([128, KP, 2, C], F8, tag="w_qc")
                dma(out=w_qc, in_=wqc2[:, :, :, :])
                w_ca = pca.tile([128, KP, 2, C], F8, tag="w_ca")
                dma(out=w_ca, in_=wca2[:, :, :, :])
                # MLP gate/up weight stream (2 mg ahead)
                wgh_t, wgl_t, wuh_t, wul_t = {}, {}, {}, {}

                def gu_fetch(mg):
                    for d, src_, nm in ((wgh_t, wgh, "gh"), (wgl_t, wgl, "gl"),
                                        (wuh_t, wuh, "uh"), (wul_t, wul, "ul")):
                        d[mg] = pca.tile([128, KP, 2, 512], F8, tag="wgu", bufs=8,
                                         name=f"w{nm}{mg}")
                        dma(out=d[mg], in_=src_[mg])

                gu_fetch(0)
                gu_fetch(1)

                # norm2 (no modulation) -> xnb2 fp8 DR-paired
                xnb2 = [pca.tile([128, 2, LQ], F8, tag=f"xn{j}", name=f"xnb{j}")
                        for j in range(KP)]
                rstd = pca.tile([1, LQ], FP32, tag="rstd", bufs=1, name="rstd_n2")
                nc.scalar.activation(out=rstd, in_=pssq_n2,
                                     func=mybir.ActivationFunctionType.Sqrt,
                                     bias=eps_c, scale=1.0 / C)
                nc.vector.reciprocal(rstd, rstd)
                rstd_bf = pca.tile([1, LQ], BF16, tag="rstd_bf", bufs=1, name="rstdb_n2")
                nc.vector.tensor_copy(rstd_bf, rstd)
                pb2 = ps.tile([128, LQ], FP32, tag="pC", bufs=2, name="pb_n2")
                nc.tensor.matmul(pb2, ones_row, rstd_bf, start=True, stop=True)
                for k in range(CT):
                    nc.vector.scalar_tensor_tensor(
                        out=xnb2[k // 2][:, k % 2, :], in0=xres[:, k, :],
                        scalar=c_n2[:, k:k + 1], in1=pb2,
                        op0=mybir.AluOpType.mult, op1=mybir.AluOpType.mult)

                def proj_dr2(out_psum, w, m, xcols, nkp=KP):
                    for kp in range(nkp):
                        nc.tensor.matmul(out_psum,
                                         w[:, kp, :, m * 128:(m + 1) * 128],
                                         xcols(kp),
                                         start=(kp == 0), stop=(kp == nkp - 1),
                                         perf_mode=DR)

                # cross q projection (m 0..1 up front, rest inside the stream)
                qcT = [pca.tile([128, LQ], BF16, tag=f"qc{m}", name=f"qcT{m}")
                       for m in range(CT)]

                def qc_proj(m):
                    pq = ps.tile([128, LQ], FP32, tag="pA", bufs=2, name=f"pqc{m}")
                    proj_dr2(pq, w_qc, m, lambda kp: xnb2[kp][:, :, :])
                    nc.scalar.copy(out=qcT[m], in_=pq)

                qc_proj(0)
                qc_proj(1)

                mark("cross_attn")
                # attention over audio (software-pipelined stream)
                CLAG = 2
                pos = {}
                pexps = {}
                pending = []

                def ca_epilogue(h):
                    def run():
                        m = h // 2
                        rs = slice((h % 2) * 64, (h % 2) * 64 + 64)
                        po = pos.pop(h)
                        rec = pp.tile([1, LQ], FP32, tag="rec", bufs=2,
                                      name=f"recc{h}")
                        nc.vector.reciprocal(rec, po[64:65, :])
                        rec_bf = pp.tile([1, LQ], BF16, tag="rec_bf", bufs=2,
                                         name=f"recbc{h}")
                        nc.vector.tensor_copy(rec_bf, rec)
                        pbc = ps.tile([64, LQ], FP32, tag="pA", bufs=2,
                                      name=f"pbcc{h}")
                        nc.tensor.matmul(pbc, ones_row[:, 0:64], rec_bf,
                                         start=True, stop=True)
                        rb_sb = pp.tile([64, LQ], BF16, tag="rb_sb", bufs=2,
                                        name=f"rbc{h}")
                        nc.vector.tensor_copy(rb_sb, pbc)
                        nc.vector.tensor_mul(att2[m // 2][rs, m % 2, :],
                                             po[0:64, :], rb_sb)
                    return run

                def ca_po(h, tp):
                    def run():
                        px = pexps.pop((h, tp))
                        for t in (2 * tp, 2 * tp + 1):
                            nc.tensor.matmul(pos[h], vcb[t][:, h, :],
                                             px[:, (t % 2) * LQ:(t % 2 + 1) * LQ],
                                             start=(t == 0), stop=(t == 3))
                    return run

                NIT = H * 2
                for g in range(NIT + 4):
                    while pending and pending[0][0] <= g:
                        pending.pop(0)[1]()
                    if g >= NIT:
                        continue
                    h, tp = divmod(g, 2)
                    m = h // 2
                    rs = slice((h % 2) * 64, (h % 2) * 64 + 64)
                    if tp == 0:
                        pos[h] = ps.tile([65, LQ], FP32, tag="pC", bufs=2,
                                         name=f"poc{h}")
                    psc = ps.tile([128, 2 * LQ], FP32, tag="pQ", bufs=2,
                                  name=f"pscc{h}_{tp}")
                    for t in (2 * tp, 2 * tp + 1):
                        nc.tensor.matmul(psc[:, (t % 2) * LQ:(t % 2 + 1) * LQ],
                                         kcT[m][rs, t * 128:(t + 1) * 128],
                                         qcT[m][rs, :], start=True, stop=True)
                    pexp = pp.tile([128, 2 * LQ], BF16, tag="pexp", bufs=3,
                                   name=f"pexpc{h}_{tp}")
                    nc.scalar.activation(out=pexp, in_=psc,
                                         func=mybir.ActivationFunctionType.Exp,
                                         scale=KSC)
                    pexps[(h, tp)] = pexp
                    pending.append((g + CLAG, ca_po(h, tp)))
                    if tp == 0 and h % 2 == 0 and h // 2 + 2 < CT:
                        qc_proj(h // 2 + 2)
                    if tp == 1:
                        pending.append((g + CLAG + 1, ca_epilogue(h)))
                        if h % 2 == 0 and 2 + h // 2 < 8:
                            pending.append((g + CLAG + 1, (lambda mg:
                                lambda: gu_fetch(mg))(2 + h // 2)))
                while pending:
                    pending.pop(0)[1]()

                mark("ca_out")
                # cross out proj + residual (no gate) + norm3 ssq (lagged)
                pssq3 = ps.tile([1, LQ], FP32, tag="pC", bufs=2, name="pssq_n3")

                def n3_ssq(m):
                    xsq = pca.tile([128, LQ], BF16, tag="xsq2", bufs=1,
                                   name=f"xsq3_{m}")
                    nc.vector.tensor_mul(xsq, xres[:, m, :], xres[:, m, :])
                    nc.tensor.matmul(pssq3, ones_col, xsq,
                                     start=(m == 0), stop=(m == CT - 1))

                for m in range(CT):
                    pco = ps.tile([128, LQ], FP32, tag="pA", bufs=2, name=f"pcao{m}")
                    proj_dr2(pco, w_ca, m, lambda kp: att2[kp][:, :, :])
                    nc.vector.tensor_add(xres[:, m, :], xres[:, m, :], pco)
                    if m >= 2:
                        n3_ssq(m - 2)
                n3_ssq(CT - 2)
                n3_ssq(CT - 1)

                mark("mlp_norm")
                # norm3 + modulation -> bf16, then hi/lo fp8 split
                xmb = [pca.tile([128, LQ], BF16, tag=f"xm{k}", name=f"xmb{k}")
                       for k in range(CT)]
                xh2 = [pca.tile([128, 2, LQ], F8, tag=f"xh{j}", name=f"xh{j}")
                       for j in range(KP)]
                xl2 = [pca.tile([128, 2, LQ], F8, tag=f"xl{j}", name=f"xl{j}")
                       for j in range(KP)]
                x64 = [pca.tile([128, 2, LQ], F8, tag=f"x6{j}", name=f"x6{j}")
                       for j in range(KP)]
                rstd3 = pca.tile([1, LQ], FP32, tag="rstd", bufs=1, name="rstd_n3")
                nc.scalar.activation(out=rstd3, in_=pssq3,
                                     func=mybir.ActivationFunctionType.Sqrt,
                                     bias=eps_c, scale=1.0 / C)
                nc.vector.reciprocal(rstd3, rstd3)
                rstd3_bf = pca.tile([1, LQ], BF16, tag="rstd_bf", bufs=1,
                                    name="rstdb_n3")
                nc.vector.tensor_copy(rstd3_bf, rstd3)
                pb3 = ps.tile([128, LQ], FP32, tag="pC", bufs=2, name="pb_n3")
                nc.tensor.matmul(pb3, ones_row, rstd3_bf, start=True, stop=True)
                for k in range(CT):
                    nc.vector.scalar_tensor_tensor(
                        out=xmb[k], in0=xres[:, k, :], scalar=w3eff[:, k:k + 1],
                        in1=pb3,
                        op0=mybir.AluOpType.mult, op1=mybir.AluOpType.mult)
                    nc.gpsimd.tensor_scalar(out=xmb[k], in0=xmb[k],
                                            scalar1=sh_ml(k), scalar2=None,
                                            op0=mybir.AluOpType.add)
                    hi = xh2[k // 2][:, k % 2, :]
                    lo = xl2[k // 2][:, k % 2, :]
                    nc.scalar.copy(out=hi, in_=xmb[k])
                    nc.vector.tensor_sub(lo, xmb[k], hi)
                    nc.scalar.activation(out=x64[k // 2][:, k % 2, :], in_=xmb[k],
                                         func=mybir.ActivationFunctionType.Identity,
                                         scale=1.0 / LOSC)

                mark("gate_up")
                # h2: fp8 DR-paired ffn activations
                h2 = [pca.tile([128, 2, LQ], F8, tag=f"h{t}", name=f"h2_{t}")
                      for t in range(FF // 256)]
                h64_2 = [pca.tile([128, 2, LQ], F8, tag=f"h6{t}", name=f"h64_{t}")
                         for t in range(FF // 256)]
                wdh_t, wdl_t = {}, {}

                def down_fetch(m):
                    wdh_t[m] = pca.tile([128, 16, 2, 128], F8, tag="wdw", bufs=4,
                                        name=f"wdh{m}")
                    dma(out=wdh_t[m], in_=wdh[m])
                    wdl_t[m] = pca.tile([128, 16, 2, 128], F8, tag="wdw", bufs=4,
                                        name=f"wdl{m}")
                    dma(out=wdl_t[m], in_=wdl[m])

                def dr_hilo(p1, wh, wl, mi, xlo=True):
                    # Xh*Wh + (X/64)*(Wl*64) [+ Xl*Wh], all at true scale
                    ms = slice(mi * 128, (mi + 1) * 128)
                    for kp in range(KP):
                        nc.tensor.matmul(p1, wh[:, kp, :, ms], xh2[kp][:, :, :],
                                         start=(kp == 0), stop=False, perf_mode=DR)
                    for kp in range(KP):
                        nc.tensor.matmul(p1, wl[:, kp, :, ms], x64[kp][:, :, :],
                                         start=False, stop=(not xlo and kp == KP - 1),
                                         perf_mode=DR)
                    if xlo:
                        for kp in range(KP):
                            nc.tensor.matmul(p1, wh[:, kp, :, ms], xl2[kp][:, :, :],
                                             start=False, stop=(kp == KP - 1),
                                             perf_mode=DR)

                for mg in range(8):
                    if mg >= 6:
                        down_fetch(mg - 6)
                    for mi in range(4):
                        pgu = ps.tile([128, 2 * LQ], FP32, tag="pQ", bufs=2,
                                      name=f"pgu{mg}_{mi}")
                        p1g = pgu[:, 0:LQ]
                        p1u = pgu[:, LQ:2 * LQ]
                        dr_hilo(p1g, wgh_t[mg], wgl_t[mg], mi)
                        sg = pca.tile([128, LQ], BF16, tag="sgb", bufs=2,
                                      name=f"sg{mg}_{mi}")
                        nc.scalar.activation(out=sg, in_=p1g,
                                             func=mybir.ActivationFunctionType.Sigmoid)
                        gbf = pca.tile([128, LQ], BF16, tag="gbf", bufs=4,
                                       name=f"gbf{mg}_{mi}")
                        nc.vector.tensor_mul(gbf, sg, p1g)
                        dr_hilo(p1u, wuh_t[mg], wul_t[mg], mi)
                        t = mg * 4 + mi
                        nc.vector.tensor_mul(h2[t // 2][:, t % 2, :], gbf, p1u)
                        h64 = h64_2[t // 2][:, t % 2, :]
                        nc.scalar.activation(
                            out=h64, in_=h2[t // 2][:, t % 2, :],
                            func=mybir.ActivationFunctionType.Identity,
                            scale=1.0 / LOSC)

                mark("down")
                # down proj: P1 = H*Wdh, P2 = H*Wdl(x64); out = (P1 + P2/64)*g + xres
                for m in range(CT):
                    if m + 2 < CT:
                        down_fetch(m + 2)
                    pd1 = ps.tile([128, LQ], FP32, tag="pA", bufs=2, name=f"pd1{m}")
                    for fp in range(16):
                        nc.tensor.matmul(pd1, wdh_t[m][:, fp, :, :],
                                         h2[fp][:, :, :],
                                         start=(fp == 0), stop=False,
                                         perf_mode=DR)
                    for fp in range(16):
                        nc.tensor.matmul(pd1, wdl_t[m][:, fp, :, :],
                                         h64_2[fp][:, :, :],
                                         start=False, stop=(fp == 15),
                                         perf_mode=DR)
                    of = pca.tile([128, LQ], FP32, tag="of", bufs=2, name=f"of{m}")
                    nc.vector.scalar_tensor_tensor(
                        out=of, in0=pd1, scalar=g_ml(m), in1=xres[:, m, :],
                        op0=mybir.AluOpType.mult, op1=mybir.AluOpType.add)
                    dma(out=outT[m * 128:(m + 1) * 128, :], in_=of)

    nc.compile()
    return nc


_ROPE_PERM = None
_SWAP_PERM = None


def _perms():
    global _ROPE_PERM, _SWAP_PERM
    if _ROPE_PERM is None:
        p = np.zeros(C, dtype=np.int64)
        s = np.zeros(C, dtype=np.int64)
        for h in range(H):
            for i in range(D // 2):
                p[h * D + i] = h * D + 2 * i               # real block
                p[h * D + D // 2 + i] = h * D + 2 * i + 1  # imag block
                s[h * D + i] = h * D + 2 * i + 1           # swapped: imag first
                s[h * D + D // 2 + i] = h * D + 2 * i
        _ROPE_PERM, _SWAP_PERM = p, s
    return _ROPE_PERM, _SWAP_PERM


def _bf(a):
    return np.ascontiguousarray(a).astype(ml_dtypes.bfloat16)


def _f8(a):
    return np.ascontiguousarray(a).astype(F8NP)


def _dr_pack(W):
    # [n_in, n_out] -> [128, n_in//256, 2, n_out]
    n_in, n_out = W.shape
    kp = n_in // 256
    return W.reshape(kp, 2, 128, n_out).transpose(2, 0, 1, 3)


def _hilo(W):
    hi = W.astype(F8NP)
    lo = ((W - hi.astype(np.float32)) * LOSC).astype(F8NP)
    return hi, lo


def _prep_shared(W_qkv, W_sa_out, W_q, W_kv, W_ca_out, W_gate, W_up, W_down,
                 adaLN_W, adaLN_b, norm1_w, norm2_w, norm3_w):
    perm, sperm = _perms()
    wq = W_qkv[:, 0:C][:, perm]
    wqs = W_qkv[:, 0:C][:, sperm]
    wk = W_qkv[:, C:2 * C][:, perm]
    wks = W_qkv[:, C:2 * C][:, sperm]
    wv = W_qkv[:, 2 * C:3 * C]

    def pack8(W):
        return _f8(_dr_pack(np.asarray(W, np.float32)))

    wgh_, wgl_ = _hilo(np.asarray(W_gate, np.float32))
    wuh_, wul_ = _hilo(np.asarray(W_up, np.float32))
    wdh_, wdl_ = _hilo(np.asarray(W_down, np.float32))

    def mlp_pack(w8):  # fp8 [C, FF] -> [8 mg][128, kp, 2, 512]
        d = _dr_pack(w8.astype(np.float32)).astype(F8NP)  # [128, 4, 2, 4096]
        return np.ascontiguousarray(d.reshape(128, KP, 2, 8, 512)
                                    .transpose(3, 0, 1, 2, 4))

    def down_pack(w8):  # fp8 [FF, C] -> [8 m][128, 16 fp, 2, 128]
        d = _dr_pack(w8.astype(np.float32)).astype(F8NP)  # [128, 16, 2, C]
        return np.ascontiguousarray(d.reshape(128, 16, 2, CT, 128)
                                    .transpose(3, 0, 1, 2, 4))

    # adaLN weight-stationary tiles: [p, k, j*128+q] = W[128k+p, 128j+q]
    wada = np.asarray(adaLN_W, np.float32).reshape(CT, 128, 48, 128)
    wadaA_h = wada[:, :, 0:16, :].transpose(1, 0, 2, 3).reshape(128, CT, 2048)
    wadaB_h = np.stack([
        wada[:, :, 16 + 4 * i:20 + 4 * i, :].transpose(1, 0, 2, 3)
        .reshape(128, CT, 512) for i in range(8)])

    sh = {
        "wq2": pack8(wq), "wqs2": pack8(wqs), "wk2": pack8(wk),
        "wks2": pack8(wks), "wv2": pack8(wv),
        "wsa2": pack8(W_sa_out), "wqc2": pack8(W_q), "wkv2": pack8(W_kv),
        "wca2": pack8(W_ca_out),
        "wgh": mlp_pack(wgh_), "wgl": mlp_pack(wgl_),
        "wuh": mlp_pack(wuh_), "wul": mlp_pack(wul_),
        "wdh": down_pack(wdh_), "wdl": down_pack(wdl_),
        "wadaA": _bf(wadaA_h), "wadaB": _bf(wadaB_h),
        "cst_base": np.concatenate([
            np.asarray(adaLN_b, np.float32).reshape(48, 128).T,
            np.asarray(norm1_w, np.float32).reshape(8, 128).T,
            np.asarray(norm2_w, np.float32).reshape(8, 128).T,
            np.asarray(norm3_w, np.float32).reshape(8, 128).T], axis=1),
    }
    return sh


def make_in_maps(x, t_mod, audio_context, freqs_cos, freqs_sin,
                 norm1_w, norm2_w, norm3_w,
                 W_qkv, W_sa_out, W_q, W_kv, W_ca_out,
                 W_gate, W_up, W_down, adaLN_W, adaLN_b):
    sh = _prep_shared(W_qkv, W_sa_out, W_q, W_kv, W_ca_out, W_gate, W_up,
                      W_down, adaLN_W, adaLN_b, norm1_w, norm2_w, norm3_w)
    cosT = np.ascontiguousarray(np.asarray(freqs_cos, np.float32).T)
    sinT = np.ascontiguousarray(np.asarray(freqs_sin, np.float32).T)

    in_maps = []
    for core in range(NCORE):
        b, j = divmod(core, 4)
        # roll the token axis so this core's own 512 tokens sit at [0, LQ)
        xT = np.roll(np.ascontiguousarray(np.asarray(x, np.float32)[b].T),
                     -j * LQ, axis=1)
        m = {k: v for k, v in sh.items() if k != "cst_base"}
        m["x_bf"] = _bf(xT)
        m["xq_f"] = np.ascontiguousarray(xT[:, 0:LQ])
        cr = np.roll(cosT, -j * LQ, axis=1)
        sr = np.roll(sinT, -j * LQ, axis=1)
        m["cs4"] = _bf(np.concatenate([cr, cr, cr, cr], axis=0))
        m["sc4"] = _bf(np.concatenate([-sr, sr, -sr, sr], axis=0))
        m["aud2"] = _f8(_dr_pack(
            np.ascontiguousarray(np.asarray(audio_context, np.float32)[b].T)))
        m["cst"] = np.ascontiguousarray(np.concatenate(
            [np.asarray(t_mod, np.float32)[b].reshape(8, 128).T,
             sh["cst_base"]], axis=1))
        in_maps.append(m)
    return in_maps


_NC_CACHE = None


def _get_nc():
    global _NC_CACHE
    if _NC_CACHE is None:
        _NC_CACHE = build_bass()
    return _NC_CACHE


def kernel(**inputs):
    nc = _get_nc()
    inputs = {k: np.asarray(v) for k, v in inputs.items()}
    in_maps = make_in_maps(**inputs)
    res = run_bass_kernel_spmd(nc, in_maps, list(range(NCORE)))
    out = np.zeros((B, L, C), np.float32)
    for core in range(NCORE):
        b, j = divmod(core, 4)
        out[b, j * LQ:(j + 1) * LQ, :] = res.results[core]["outT"].T
    return out
